# revision 1
# baseline (speedup 1.0000x reference)
"""Contrastive-loss kernel for Trainium2 (Bass/Tile), 8-core SPMD.

Reference semantics (B=4, N=4096, D=128, T=0.1):
    emb_n = emb / max(||emb||, 1e-12)
    pos_sim[b,n] = dot(emb_n[b,n], emb_n[b, pos_idx[b,n]]) / T
    loss = mean(softplus(-pos_sim)) + mean(softplus(neg_sim))

The reference's full [B,N,N] similarity matrix is only read at one column per
row for pos and one for neg, so the kernel needs just two gathered dot
products per row.  Each of the 8 cores handles half the rows of one batch
element: it loads its 2048 "own" rows contiguously (bf16), gathers the 2048
pos / 2048 neg partner rows from the 4096-row table with dma_gather
(256B/row descriptors, single_packet=False — required above ~1024 rows),
and computes

    z    = -+ dot(own, g) * exp(-0.5*(ln ssq_own + ln ssq_g)) / T
    part = softplus(z) = ln(exp(z) + 1)

with [128, slice] bf16 ops: products/squares split between DVE (2x bf16 mode)
and ACT, group-of-128 row sums as a pairwise-add tree plus a short reduce
(pos dots negated in the tree so one Exp serves both directions), and the +1
folded into the final Ln's bias.  All ACT functions (Square/Exp/Ln) are
pinned to one activation table to avoid table-reload ping-pong.  Output is a
[128, 1] per-partition partial-sum tile per core; the host sums / (B*N).
"""

import numpy as np

B, N, D = 4, 4096, 128
NCORES = 8
HALF = N // 2        # rows per core
CHUNK = HALF // 128  # 16 column-groups of 128
TEMP = 0.1
NSPLIT = 2           # pipeline slices per direction
TREE_LEVELS = 3

_PROG = None


def _pin_act_table(table_name="natural_log_exp_and_others"):
    """Make Square/Ln/Exp resolve only to `table_name` so the act-table-load
    pass emits a single table load instead of ping-ponging between tables.
    Keeps the table dict size and order intact (act_func_set_id indexes the
    full act_info.json list)."""
    import functools
    import concourse.hw_specs as hw_specs
    import concourse.bacc as bacc
    import concourse.mybir as mybir

    if getattr(_pin_act_table, "_done", False):
        return
    orig = hw_specs.get_activation_tables
    AF = mybir.ActivationFunctionType
    pinned = {AF.Square, AF.Ln, AF.Exp}

    @functools.cache
    def patched(arch):
        return {k: (v if k == table_name else v - pinned)
                for k, v in orig(arch).items()}

    hw_specs.get_activation_tables = patched
    bacc.get_activation_tables = patched
    _pin_act_table._done = True


def _build_program():
    import concourse.bacc as bacc
    import concourse.tile as tile
    import concourse.mybir as mybir

    _pin_act_table()

    f32 = mybir.dt.float32
    bf16 = mybir.dt.bfloat16
    i16 = mybir.dt.int16
    mult = mybir.AluOpType.mult
    add = mybir.AluOpType.add
    X = mybir.AxisListType.X
    AF = mybir.ActivationFunctionType

    nc = bacc.Bacc("TRN2", target_bir_lowering=False)

    table = nc.dram_tensor("table", [N, D], bf16, kind="ExternalInput")
    own = nc.dram_tensor("own", [128, HALF], bf16, kind="ExternalInput")
    posi = nc.dram_tensor("pos_idx", [128, 128], i16, kind="ExternalInput")
    negi = nc.dram_tensor("neg_idx", [128, 128], i16, kind="ExternalInput")
    out = nc.dram_tensor("partial", [128, 1], f32, kind="ExternalOutput")

    SC = CHUNK // NSPLIT   # column-groups per slice
    SW = SC * D            # free-dim elements per slice
    NIDX = HALF // NSPLIT  # rows per gather slice

    with tile.TileContext(nc) as tc:
        with tc.tile_pool(name="p", bufs=1) as pool:
            idx_t = {}
            for name, src in (("pos", posi), ("neg", negi)):
                t = pool.tile([128, 128], i16, tag=f"idx{name}")
                nc.sync.dma_start(out=t[:], in_=src[:])
                idx_t[name] = t
            own_t = pool.tile([128, HALF], bf16)
            nc.sync.dma_start(out=own_t[:], in_=own[:])

            gath = {"pos": [], "neg": []}
            for s in range(NSPLIT):
                for name in ("pos", "neg"):
                    g = pool.tile([128, SW], bf16, tag=f"g{name}{s}")
                    nc.gpsimd.dma_gather(
                        out_ap=g[:].rearrange("p (c d) -> p c d", d=D),
                        in_ap=table[:],
                        idxs_ap=idx_t[name][:, s * (128 // NSPLIT):(s + 1) * (128 // NSPLIT)],
                        num_idxs=NIDX,
                        num_idxs_reg=NIDX,
                        elem_size=D,
                        single_packet=False,
                    )
                    gath[name].append(g)

            def group_sum(src_ap, tag, out_ap=None, negate=None):
                """[128, SC*D] bf16 AP -> [128, SC] f32 row-group sums."""
                w = D
                cur = src_ap
                for lvl in range(TREE_LEVELS):
                    t = pool.tile([128, SC * (w // 2)], bf16, tag=f"{tag}l{lvl}")
                    v = cur.rearrange("p (c d) -> p c d", d=w)
                    nc.vector.tensor_tensor(
                        out=t[:].rearrange("p (c d) -> p c d", d=w // 2),
                        in0=v[:, :, 0:w // 2], in1=v[:, :, w // 2:w], op=add)
                    cur = t[:]
                    w //= 2
                if out_ap is None:
                    r = pool.tile([128, SC], f32, tag=f"{tag}r")
                    out_ap = r[:]
                nc.vector.tensor_reduce(
                    out=out_ap, in_=cur.rearrange("p (c d) -> p c d", d=w),
                    axis=X, op=add, negate=negate)
                return out_ap

            def ssq_of(src_ap, tag, out_ap=None):
                sq = pool.tile([128, SC * D], bf16, tag=f"{tag}sq")
                nc.scalar.square(sq[:], src_ap)
                return group_sum(sq[:], tag, out_ap=out_ap)

            out_t = pool.tile([128, 1], f32)
            accs = []
            for s in range(NSPLIT):
                o_ap = own_t[:, s * SW:(s + 1) * SW]
                ssq_own = ssq_of(o_ap, f"so{s}")
                # both-direction tiles: pos in cols 0:SC, neg in SC:2SC.
                # pos dot negated in its tree so one Exp(x/T) serves both
                # softplus(-pos_sim) and softplus(+neg_sim).
                ssq_b = pool.tile([128, 2 * SC], f32, tag=f"ssqb{s}")
                dot_b = pool.tile([128, 2 * SC], f32, tag=f"dotb{s}")
                for i, name in enumerate(("pos", "neg")):
                    g = gath[name][s]
                    ssq_of(g[:], f"s{name}{s}",
                           out_ap=ssq_b[:, i * SC:(i + 1) * SC])
                    prod = pool.tile([128, SC * D], bf16, tag=f"d{name}{s}pr")
                    nc.vector.tensor_tensor(
                        out=prod[:], in0=o_ap, in1=g[:], op=mult)
                    group_sum(prod[:], f"d{name}{s}",
                              out_ap=dot_b[:, i * SC:(i + 1) * SC],
                              negate=(name == "pos"))
                sprod = pool.tile([128, 2 * SC], f32, tag=f"sprod{s}")
                for i in range(2):
                    nc.vector.tensor_tensor(
                        out=sprod[:, i * SC:(i + 1) * SC],
                        in0=ssq_own, in1=ssq_b[:, i * SC:(i + 1) * SC],
                        op=mult)
                lnp = pool.tile([128, 2 * SC], f32, tag=f"lnp{s}")
                nc.scalar.activation(lnp[:], sprod[:], AF.Ln)
                rinv = pool.tile([128, 2 * SC], f32, tag=f"rinv{s}")
                nc.scalar.activation(rinv[:], lnp[:], AF.Exp, scale=-0.5)
                cosz = pool.tile([128, 2 * SC], f32, tag=f"cosz{s}")
                nc.vector.tensor_tensor(
                    out=cosz[:], in0=dot_b[:], in1=rinv[:], op=mult)
                ez = pool.tile([128, 2 * SC], f32, tag=f"ez{s}")
                nc.scalar.activation(ez[:], cosz[:], AF.Exp, scale=1.0 / TEMP)
                sp = pool.tile([128, 2 * SC], f32, tag=f"sp{s}")
                if NSPLIT == 1:
                    a = out_t[:, 0:1]
                else:
                    at = pool.tile([128, 1], f32, tag=f"acc{s}")
                    a = at[:]
                    accs.append(at)
                # softplus(z) = ln(exp(z) + 1): the +1 folds into Ln's bias
                nc.scalar.activation(sp[:], ez[:], AF.Ln, bias=1.0, accum_out=a)

            if accs:
                t = accs[0]
                for s in range(1, len(accs)):
                    t2 = pool.tile([128, 1], f32, tag=f"accsum{s}")
                    nc.vector.tensor_tensor(
                        out=t2[:], in0=t[:], in1=accs[s][:], op=add)
                    t = t2
                nc.vector.tensor_copy(out_t[:, 0:1], t[:])

            nc.sync.dma_start(out=out[:], in_=out_t[:])

    nc.compile()
    return nc


def _get_program():
    global _PROG
    if _PROG is None:
        _PROG = _build_program()
    return _PROG


def _wrap_idx(rows):
    """Host-side index layout for dma_gather.

    rows[n] is the partner row for local own-row n (n = p*CHUNK + t in the
    on-chip layout).  Each gather slice s covers chunks [s*SC, (s+1)*SC) and
    reads idx tile columns [s*(128/NSPLIT), ...).  Within a slice,
    dma_gather places gathered row i at partition i%128, chunk i//128, and
    the Q7 cores read the slice's index columns wrapped into 16 partitions
    (idxs[pi, col] = unwrapped[col*16 + pi]) replicated across the 8
    16-partition groups.
    """
    cols = []
    sc = CHUNK // NSPLIT
    ncol = 128 // NSPLIT
    for s in range(NSPLIT):
        # rows for slice s in gather order: unwrapped[t*128 + p] =
        # rows[p*CHUNK + s*sc + t]
        sl = rows.reshape(128, CHUNK)[:, s * sc:(s + 1) * sc]  # [128, sc]
        unwrapped = sl.T.ravel()                               # [sc*128]
        cols.append(unwrapped.reshape(ncol, 16).T)             # [16, ncol]
    wrapped = np.concatenate(cols, axis=1).astype(np.int16)    # [16, 128]
    return np.tile(wrapped, (8, 1))                            # [128, 128]


def _shard_inputs(embeddings, positive_pairs, negative_pairs):
    import ml_dtypes

    emb = np.asarray(embeddings, dtype=np.float32)
    emb_bf = emb.astype(ml_dtypes.bfloat16)
    pos = np.asarray(positive_pairs).reshape(B, N)
    neg = np.asarray(negative_pairs).reshape(B, N)

    in_maps = []
    for c in range(NCORES):
        b, h = divmod(c, 2)
        own_rows = emb_bf[b, h * HALF:(h + 1) * HALF]       # [HALF, D]
        in_maps.append({
            "table": np.ascontiguousarray(emb_bf[b]),
            "own": np.ascontiguousarray(own_rows.reshape(128, CHUNK * D)),
            "pos_idx": _wrap_idx(pos[b, h * HALF:(h + 1) * HALF]),
            "neg_idx": _wrap_idx(neg[b, h * HALF:(h + 1) * HALF]),
        })
    return in_maps


def kernel(embeddings, positive_pairs, negative_pairs):
    from concourse.bass_utils import run_bass_kernel_spmd

    nc = _get_program()
    in_maps = _shard_inputs(embeddings, positive_pairs, negative_pairs)
    res = run_bass_kernel_spmd(nc, in_maps, core_ids=list(range(NCORES)))
    total = sum(r["partial"].astype(np.float64).sum() for r in res.results)
    return np.float32(total / (B * N))

